# revision 25
# baseline (speedup 1.0000x reference)
"""Trainium2 Bass kernel for nn_ChannelWiseQuantumKernel.

Method: the per-position data RY gates are converted to diagonal phase gates
via RY(t) = (SH) RZ(t) (SH)^dag, so the circuit per patch becomes
    y <- G_pos (e^{i a} . y),   a_i = 0.5 * sum_ch (2 b_ch(i) - 1) theta_ch
with fixed 16x16 complex G_pos precomputed on host from the (tiny) weights.

Key structural trick ("g-trick"): patches overlap, so the phase angles for
position (dy,dx) at patch (py,px) are functions of PIXEL (py+dy, px+dx) only.
We compute a per-pixel sign-combination tensor g (128 rows x 4096 pixels) with
ONE small matmul per core, one Sin activation pass, and one cube
(triple-angle range reduction) pass; every position then reads the resulting
phase tensor w via a shifted 2-D window access pattern.  Per position the
device does only: ONE VectorE multiply (w-window x state, extended layout
[Re;Im;Im;Re] x [c;c;s;s] giving all 4 real products) and ONE 128x128
tensor-engine matmul.  Position 0 needs no multiply at all (the matmul's
moving operand is the w-window itself; the initial state is folded into B0).
State lives in PSUM ping-pong banks; patches stream in patch-row chunks.
Measurement: Square (ScalarE) + one matmul against a signed-sum matrix.
"""

import sys

sys.path.insert(0, "/opt/trn_rl_repo")

import numpy as np

import concourse.bacc as bacc
import concourse.bass as bass
import concourse.tile as tile
from concourse import mybir
from concourse.bass_utils import run_bass_kernel_spmd

# ---------------- problem constants ----------------
IN_CH = 4
KSZ = 3
NPOS = 9
DIM = 16
B = 16
HW = 64
OH = HW - KSZ + 1  # 62
P = OH * OH  # 3844 patches per image
N_CORES = 8
IMGS_PER_CORE = B // N_CORES  # 2 (the two partition-chunks)
F = P
NPIX = HW * HW  # 4096
ROWCHUNK = 16  # patch rows per chunk (chunk = ROWCHUNK*62 <= 992 cols)

MM_DT = mybir.dt.float32r
F16 = mybir.dt.float16

# fp32 wmats: SGN8 (8 rows x 128 cols) + BIAS column
COL_SGN = 0
COL_BIAS = 128
WM_COLS = 129

# fp16 stage-matrix array: per kernel B0 (128) + W1..W7 (7*128) + W8 (64),
# then ZL2 (128 x 16) for the packed two-kernel measurement.
K16COLS = 128 + 7 * 128 + 64  # 1088
COL_ZL = 2 * K16COLS  # 2176
WM16_COLS = COL_ZL + 16  # 2192

# Per-stream multiply mode: 'A' = direct DVE fp32 (PSUM operand, 1x),
# 'B' = ScalarE copy PSUM->SBUF fp16 then DVE tensor_tensor at 2x,
# 'C' = ScalarE copy then GPSIMD tensor_tensor (SBUF-only engine).
C_STREAMS = {(1, 0), (4, 0), (7, 1)}
B_STREAMS = {(2, 0), (5, 0), (8, 0), (3, 1), (6, 1), (8, 1)}


def _mode(pos, k):
    if (pos, k) in C_STREAMS:
        return "C"
    if (pos, k) in B_STREAMS:
        return "B"
    return "A"

# ---------------- host-side constant math (weights-only, O(1)) ----------------
_H = np.array([[1, 1], [1, -1]], dtype=np.complex128) / np.sqrt(2)
_S = np.array([[1, 0], [0, 1j]], dtype=np.complex128)
_A1 = _S @ _H


def _kron_n(mats):
    out = np.array([[1.0 + 0j]])
    for m in mats:
        out = np.kron(out, m)
    return out


_AA = _kron_n([_A1] * IN_CH)
_U1 = _A1.conj().T @ (np.array([1.0, 1.0]) / np.sqrt(2))
_YINIT = _kron_n([_U1.reshape(2, 1)] * IN_CH).reshape(DIM)

_SGN = np.array(
    [[2 * ((i >> (3 - ch)) & 1) - 1 for i in range(DIM)] for ch in range(IN_CH)],
    dtype=np.float64,
)
_SIGMA = np.array(
    [[1 - 2 * ((i >> (3 - q)) & 1) for i in range(DIM)] for q in range(IN_CH)],
    dtype=np.float64,
)


def _rx(t):
    c, s = np.cos(t / 2), np.sin(t / 2)
    return np.array([[c, -1j * s], [-1j * s, c]])


def _ry(t):
    c, s = np.cos(t / 2), np.sin(t / 2)
    return np.array([[c, -s], [s, c]])


def _rz(t):
    e = np.exp(-0.5j * t)
    return np.array([[e, 0], [0, np.conj(e)]])


def _embed(U, q):
    mats = [np.eye(2, dtype=complex)] * IN_CH
    mats[q] = U
    return _kron_n(mats)


def _cx(cq, tq):
    M = np.zeros((DIM, DIM), dtype=complex)
    for i in range(DIM):
        bits = [(i >> (3 - q)) & 1 for q in range(4)]
        j = i
        if bits[cq] == 1:
            bits2 = bits.copy()
            bits2[tq] ^= 1
            j = sum(b << (3 - q) for q, b in enumerate(bits2))
        M[j, i] = 1
    return M


def _build_G(w_flat):
    w = np.float64(w_flat).reshape(NPOS, 1, IN_CH, 3)
    Gs = []
    for pos in range(NPOS):
        U = np.eye(DIM, dtype=complex)
        for q in range(IN_CH):
            R = _rz(w[pos, 0, q, 2]) @ _ry(w[pos, 0, q, 1]) @ _rx(w[pos, 0, q, 0])
            U = _embed(R, q) @ U
        for q in range(IN_CH - 1):
            U = _cx(q, q + 1) @ U
        U = _cx(IN_CH - 1, 0) @ U
        Gs.append(_AA.conj().T @ U @ _AA)
    G8f = _AA @ Gs[8]
    return Gs, G8f


def _wc_of(G, bouts):
    """Extended-state transition block: rows = m blocks [cR, cI, sI, sR],
    cols = output ext blocks listed in `bouts` (0/3 = Re, 1/2 = Im)."""
    Gr, Gi = G.real, G.imag
    Wc = np.zeros((64, 16 * len(bouts)))
    for o, bout in enumerate(bouts):
        re_out = bout in (0, 3)
        for j in range(DIM):
            col = o * 16 + j
            if re_out:
                Wc[0:16, col] = Gr[j]
                Wc[16:32, col] = -Gi[j]
                Wc[32:48, col] = -Gr[j]
                Wc[48:64, col] = -Gi[j]
            else:
                Wc[0:16, col] = Gi[j]
                Wc[16:32, col] = Gr[j]
                Wc[32:48, col] = -Gi[j]
                Wc[48:64, col] = Gr[j]
    return Wc


def _blockdiag2(M):
    Z = np.zeros((128, 2 * M.shape[1]), dtype=np.float32)
    Z[:64, : M.shape[1]] = M
    Z[64:, M.shape[1] :] = M
    return Z


# Triple-angle range reduction: the ScalarE Sin spline is only valid on
# [-pi, pi] but alpha reaches ~5. We compute v = sin(alpha/3 + delta)
# (delta = pi/6 for cos rows, 0 for sin rows), then w = (v^2 - 3/4) * v
# = -Phi/4, and absorb the -4 into the stage matrices.
_STAGE_SCALE = -4.0


def _build_wmats(weights):
    """(128, WM_COLS) fp32: SGN8 sign matrix + Sin bias column."""
    wm = np.zeros((128, WM_COLS), dtype=np.float32)
    # SGN8 (8, 128): row 4m+ch -> g rows 64m + 16b + i, entries _SGN[ch,i]/6
    # (emits alpha/3 directly).
    for m in range(IMGS_PER_CORE):
        for ch in range(IN_CH):
            for b in range(4):
                for i in range(DIM):
                    wm[4 * m + ch, COL_SGN + 64 * m + 16 * b + i] = _SGN[ch, i] / 6.0
    # BIAS (128, 1): pi/6 on cos rows (blocks 0,1), 0 on sin rows (blocks 2,3)
    bias = np.zeros(128, dtype=np.float32)
    for chunk in range(2):
        bias[chunk * 64 : chunk * 64 + 32] = np.pi / 6
    wm[:, COL_BIAS] = bias
    return wm


def _build_wmats16(weights):
    """(128, WM16_COLS) fp16: B0, W1..W7, W8 per kernel + ZL2."""
    wm = np.zeros((128, WM16_COLS), dtype=np.float16)
    for k in range(2):
        Gs, G8f = _build_G(weights[k])
        G0c = Gs[0] @ np.diag(_YINIT)
        # pos-0 matrix: rhs is the w-window itself ([c;c;s;s] blocks)
        B0c = np.zeros((64, 64))
        G0r, G0i = G0c.real, G0c.imag
        for bout in range(4):
            re_out = bout in (0, 3)
            for j in range(DIM):
                col = bout * 16 + j
                if re_out:
                    B0c[0:16, col] = G0r[j]
                    B0c[32:48, col] = -G0i[j]
                else:
                    B0c[0:16, col] = G0i[j]
                    B0c[32:48, col] = G0r[j]
        base = k * K16COLS
        wm[:, base : base + 128] = (_STAGE_SCALE * _blockdiag2(B0c)).astype(
            np.float16
        )
        for p in range(1, 8):
            wm[:, base + p * 128 : base + (p + 1) * 128] = (
                _STAGE_SCALE * _blockdiag2(_wc_of(Gs[p], [0, 1, 2, 3]))
            ).astype(np.float16)
        wm[:, base + 1024 : base + 1088] = (
            _STAGE_SCALE * _blockdiag2(_wc_of(G8f, [0, 1]))
        ).astype(np.float16)
    wm[:, COL_ZL : COL_ZL + 16] = _zl2().astype(np.float16)
    return wm


def _zl2():
    """ZL2 (128, 16) packed 2-kernel measurement matrix. Row
    64k + 32m + 16h + i, col 8k + 4m + q = SIGMA[q, i]."""
    zl = np.zeros((128, 16), dtype=np.float32)
    for k in range(2):
        for m in range(IMGS_PER_CORE):
            for h in range(2):
                for q in range(IN_CH):
                    for i in range(DIM):
                        zl[64 * k + 32 * m + 16 * h + i, 8 * k + 4 * m + q] = _SIGMA[
                            q, i
                        ]
    return zl


def _build_x2(x):
    """x (16, 4, 64, 64) -> per-core pixel arrays (8, 4096):
    rows = [img0 ch0..3, img1 ch0..3]."""
    xf = np.ascontiguousarray(x, dtype=np.float32).reshape(B, IN_CH, NPIX)
    return [
        np.ascontiguousarray(xf[2 * c : 2 * c + 2].reshape(8, NPIX))
        for c in range(N_CORES)
    ]


# ---------------- custom fused DVE op: w = ((v^2 - 3/4) * v) * y ------------
_CUBE_OP = None


def _register_cube_mul():
    """Register the fused triple-angle multiply as a custom DVE op."""
    global _CUBE_OP
    if _CUBE_OP is not None:
        return _CUBE_OP
    import concourse.dve_ops as dve_ops

    for o in dve_ops.OPS:
        if o.name == "CUBE_MUL_ANT":
            _CUBE_OP = o
            return o
    from concourse.dve_ops import DveOp
    from concourse.dve_spec import C0, Spec, Src0, Src1, lower
    from concourse.dve_uop import DveOpSpec

    body = ((Src0 * Src0 - C0) * Src0) * Src1
    spec = Spec(
        body=body,
        reference=lambda in0, in1, c0, c1, c2: (
            ((in0.astype(np.float32) * in0 - c0) * in0) * in1
        ),
    )
    row = max(dve_ops._SUB_OPCODE_FOR_NAME.values()) + 1
    shas = {}
    for ver in ("v3", "v4"):
        uops = lower(spec, ver=ver)
        shas[ver] = DveOpSpec(
            name="CUBE_MUL_ANT", opcode=row, uops=uops, rd1_en=True
        ).sha(ver)
    op = DveOp("CUBE_MUL_ANT", spec, subdim=False, uops_sha=shas)
    dve_ops.OPS.append(op)
    dve_ops._SUB_OPCODE_FOR_NAME[op.name] = row
    dve_ops.CUSTOM_DVE_SPECS[op.name] = spec
    _CUBE_OP = op
    return op


# ---------------- device program ----------------
_PROGRAM_CACHE = {}


def _build_program(mm_dt):
    key = str(mm_dt)
    if key in _PROGRAM_CACHE:
        return _PROGRAM_CACHE[key]

    nc = bacc.Bacc("TRN2", target_bir_lowering=False, debug=False)
    x2_d = nc.dram_tensor("xpix", [8, NPIX], mm_dt, kind="ExternalInput").ap()
    wm_d = nc.dram_tensor(
        "wmats", [128, WM_COLS], mm_dt, kind="ExternalInput"
    ).ap()
    wm16_d = nc.dram_tensor(
        "wmats16", [128, WM16_COLS], mybir.dt.float16, kind="ExternalInput"
    ).ap()
    z_d = nc.dram_tensor(
        "zout", [2, 8, F], mybir.dt.float32, kind="ExternalOutput"
    ).ap()

    f32 = mybir.dt.float32
    CUBE = _register_cube_mul()

    # patch-row chunks: (py0, nrows)
    chunks = []
    py0 = 0
    while py0 < OH:
        chunks.append((py0, min(ROWCHUNK, OH - py0)))
        py0 += ROWCHUNK
    nch = len(chunks)

    with tile.TileContext(nc) as tc:
        from contextlib import ExitStack

        with ExitStack() as ctx:
            const_pool = ctx.enter_context(tc.tile_pool(name="const", bufs=1))
            m16_pool = ctx.enter_context(tc.tile_pool(name="m16", bufs=4))
            y16_pool = ctx.enter_context(tc.tile_pool(name="y16", bufs=4))
            sq_pool = ctx.enter_context(tc.tile_pool(name="sq", bufs=2))
            zs_pool = ctx.enter_context(tc.tile_pool(name="zs", bufs=2))
            # two PSUM pools, 2 bufs x (128,1024) fp32 = 2 banks each -> 8 banks
            ypA = ctx.enter_context(tc.tile_pool(name="ypA", bufs=2, space="PSUM"))
            ypB = ctx.enter_context(tc.tile_pool(name="ypB", bufs=2, space="PSUM"))
            yp = [ypA, ypB]

            wm_sb = const_pool.tile([128, WM_COLS], mm_dt)
            nc.sync.dma_start(wm_sb[:], wm_d[:])
            wm16_sb = const_pool.tile([128, WM16_COLS], F16)
            nc.sync.dma_start(wm16_sb[:], wm16_d[:])
            x2_sb = const_pool.tile([8, NPIX], mm_dt)
            nc.sync.dma_start(x2_sb[:], x2_d[:])
            ones_sb = const_pool.tile([128, 1024], f32)
            nc.vector.memset(ones_sb[:], 1.0)
            v_sb = const_pool.tile([128, NPIX], f32)
            w_sb = const_pool.tile([128, NPIX], mm_dt)
            w16_sb = const_pool.tile([128, NPIX], F16)
            w16o_sb = const_pool.tile([128, NPIX], F16)

            sgn_l = wm_sb[0:8, COL_SGN : COL_SGN + 128]
            z_l2 = wm16_sb[:, COL_ZL : COL_ZL + 16]
            bias_col = wm_sb[:, COL_BIAS : COL_BIAS + 1].bitcast(f32)

            def w16_ap(k, p):
                base = k * K16COLS
                if p == 0:
                    return wm16_sb[:, base : base + 128]
                if p < 8:
                    return wm16_sb[:, base + p * 128 : base + (p + 1) * 128]
                return wm16_sb[:, base + 1024 : base + 1088]

            def w_win(ci, pos):
                py0, nr = chunks[ci]
                dy, dx = divmod(pos, 3)
                ap = w_sb[:].rearrange("p (r c) -> p r c", c=HW)
                ap = ap[:, py0 + dy : py0 + dy + nr, dx : dx + OH]
                return ap.rearrange("p (s r) c -> p s r c", s=2)

            def w16_win(ci, pos, half=None):
                py0, nr = chunks[ci]
                dy, dx = divmod(pos, 3)
                if dx % 2 == 1:
                    srcw, dx = w16o_sb, dx - 1
                else:
                    srcw = w16_sb
                ap = srcw[:].rearrange("p (r c) -> p r c", c=HW)
                if half is None:
                    ap = ap[:, py0 + dy : py0 + dy + nr, dx : dx + OH]
                    return ap.rearrange("p (s r) c -> p s r c", s=2)
                n0 = nr // 2
                r0 = py0 + dy + half * n0
                return ap[:, r0 : r0 + n0, dx : dx + OH]

            def y_seg(t, nr, rows=128):
                # segmented 2-bank PSUM view: (rows, 2, halfwidth)
                hw_ = (nr // 2) * OH
                ap = t[0:rows].rearrange("p (s c) -> p s c", c=512)
                return ap[:, 0:2, 0:hw_]

            # ---- prologue: g -> v -> w over the full pixel grid ----
            for j in range(NPIX // 1024):
                g_ps = ypA.tile([128, 1024], f32, tag="ypA", name=f"g{j}")
                for h in range(2):
                    nc.tensor.matmul(
                        g_ps[:, h * 512 : (h + 1) * 512],
                        sgn_l,
                        x2_sb[:, j * 1024 + h * 512 : j * 1024 + (h + 1) * 512],
                        start=True,
                        stop=True,
                    )
                nc.scalar.activation(
                    v_sb[:, j * 1024 : (j + 1) * 1024],
                    g_ps[:],
                    mybir.ActivationFunctionType.Sin,
                    bias=bias_col,
                    scale=1.0,
                )
                nc.vector._custom_dve(
                    CUBE,
                    out=w_sb[:, j * 1024 : (j + 1) * 1024],
                    in0=v_sb[:, j * 1024 : (j + 1) * 1024],
                    in1=ones_sb[:],
                    s0=0.75,
                )
            # fp16 copies of w for the 2x-mode multiply streams (and an
            # odd-pixel-shifted copy so window starts stay 4B-aligned).
            nc.vector.tensor_copy(w16_sb[:], w_sb[:].bitcast(f32))
            nc.vector.tensor_copy(
                w16o_sb[:, 0 : NPIX - 1], w_sb[:, 1:NPIX].bitcast(f32)
            )

            chain = {}

            def do_state(ci, pos):
                py0, nr = chunks[ci]
                hw_ = (nr // 2) * OH
                cur = chain.setdefault(ci, [None, None])
                # pos 8: both kernels' final [Re;Im] states share one tile
                # (k0 -> rows 0:64, k1 -> rows 64:128) for a packed measure.
                y8 = None
                if pos == 8:
                    y8 = ypA.tile([128, 1024], f32, tag="ypA", name=f"y8_{ci}")
                for k in range(2):
                    if pos == 8:
                        y_new = y8
                        row0 = 64 * k
                    else:
                        y_new = yp[k].tile(
                            [128, 1024],
                            f32,
                            tag=f"yp{'AB'[k]}",
                            name=f"y{ci}_{k}_{pos}",
                        )
                        row0 = 0
                    rows = 64 if pos == 8 else 128
                    if pos == 0:
                        for h in range(2):
                            nc.tensor.matmul(
                                y_new[row0 : row0 + rows, h * 512 : h * 512 + hw_],
                                w16_ap(k, 0),
                                w16_win(ci, 0, h),
                                start=True,
                                stop=True,
                            )
                    else:
                        mode = _mode(pos, k)
                        m16 = m16_pool.tile([128, 1024], F16, tag="m16")
                        if mode == "A":
                            nc.vector.tensor_mul(
                                m16[:, : 2 * hw_], w_win(ci, pos), y_seg(cur[k], nr)
                            )
                        else:
                            y16 = y16_pool.tile([128, 1024], F16, tag="y16")
                            nc.scalar.copy(y16[:, : 2 * hw_], y_seg(cur[k], nr))
                            eng = nc.vector if mode == "B" else nc.gpsimd
                            eng.tensor_mul(
                                m16[:, : 2 * hw_], w16_win(ci, pos), y16[:, : 2 * hw_]
                            )
                        for h in range(2):
                            nc.tensor.matmul(
                                y_new[row0 : row0 + rows, h * 512 : h * 512 + hw_],
                                w16_ap(k, pos),
                                m16[:, h * hw_ : (h + 1) * hw_],
                                start=True,
                                stop=True,
                            )
                    cur[k] = y_new

            def do_meas(ci, _pos):
                py0, nr = chunks[ci]
                hw_ = (nr // 2) * OH
                c0 = py0 * OH
                y8 = chain.pop(ci)[0]
                sq = sq_pool.tile([128, 1024], F16, tag="sq")
                nc.scalar.activation(
                    sq[:, : 2 * hw_],
                    y_seg(y8, nr),
                    mybir.ActivationFunctionType.Square,
                )
                zq = ypB.tile([128, 1024], f32, tag="ypB", name=f"zq{ci}")
                for h in range(2):
                    nc.tensor.matmul(
                        zq[0:16, h * 512 : h * 512 + hw_],
                        z_l2,
                        sq[:, h * hw_ : (h + 1) * hw_],
                        start=True,
                        stop=True,
                    )
                zs = zs_pool.tile([16, 1024], f32, tag="zs")
                nc.scalar.copy(zs[0:16, : 2 * hw_], y_seg(zq, nr, rows=16))
                nc.sync.dma_start(
                    z_d[:, :, c0 : c0 + 2 * hw_].rearrange("a b c -> (a b) c"),
                    zs[0:16, : 2 * hw_],
                )

            # merged time-ordered emission: chains pipelined across chunks
            OFF = 3.0
            events = []
            for ci in range(nch):
                t0 = ci * OFF
                for pos in range(NPOS):
                    events.append((t0 + pos, 2, ci, pos, do_state))
                events.append((t0 + NPOS, 3, ci, 0, do_meas))
            events.sort(key=lambda e: (e[0], e[1], e[2]))
            for _t, _kind, ci, pos, fn in events:
                fn(ci, pos)

    nc.compile()
    _PROGRAM_CACHE[key] = nc
    return nc


# ---------------- entry point ----------------
def kernel(x, weights):
    x = np.asarray(x, dtype=np.float32)
    weights = np.asarray(weights, dtype=np.float32)
    wm = _build_wmats(weights)
    wm16 = _build_wmats16(weights)
    x2s = _build_x2(x)

    nc = _build_program(MM_DT)
    in_maps = [
        {"xpix": x2s[c], "wmats": wm, "wmats16": wm16} for c in range(N_CORES)
    ]
    res = run_bass_kernel_spmd(nc, in_maps, list(range(N_CORES)))

    out = np.zeros((B, 2 * IN_CH, OH, OH), dtype=np.float32)
    for c in range(N_CORES):
        z = np.asarray(res.results[c]["zout"])  # (2, 8, F)
        for k in range(2):
            for chunk in range(2):
                b = 2 * c + chunk
                out[b, k * 4 : k * 4 + 4] = z[k, chunk * 4 : chunk * 4 + 4].reshape(
                    IN_CH, OH, OH
                )
    return out


# revision 27
# speedup vs baseline: 1.0324x; 1.0324x over previous
"""Trainium2 Bass kernel for nn_ChannelWiseQuantumKernel.

Method: the per-position data RY gates are converted to diagonal phase gates
via RY(t) = (SH) RZ(t) (SH)^dag, so the circuit per patch becomes
    y <- G_pos (e^{i a} . y),   a_i = 0.5 * sum_ch (2 b_ch(i) - 1) theta_ch
with fixed 16x16 complex G_pos precomputed on host from the (tiny) weights.

Key structural trick ("g-trick"): patches overlap, so the phase angles for
position (dy,dx) at patch (py,px) are functions of PIXEL (py+dy, px+dx) only.
We compute a per-pixel sign-combination tensor g (128 rows x 4096 pixels) with
ONE small matmul per core, one Sin activation pass, and one cube
(triple-angle range reduction) pass; every position then reads the resulting
phase tensor w via a shifted 2-D window access pattern.  Per position the
device does only: ONE VectorE multiply (w-window x state, extended layout
[Re;Im;Im;Re] x [c;c;s;s] giving all 4 real products) and ONE 128x128
tensor-engine matmul.  Position 0 needs no multiply at all (the matmul's
moving operand is the w-window itself; the initial state is folded into B0).
State lives in PSUM ping-pong banks; patches stream in patch-row chunks.
Measurement: Square (ScalarE) + one matmul against a signed-sum matrix.
"""

import sys

sys.path.insert(0, "/opt/trn_rl_repo")

import numpy as np

import concourse.bacc as bacc
import concourse.bass as bass
import concourse.tile as tile
from concourse import mybir
from concourse.bass_utils import run_bass_kernel_spmd

# ---------------- problem constants ----------------
IN_CH = 4
KSZ = 3
NPOS = 9
DIM = 16
B = 16
HW = 64
OH = HW - KSZ + 1  # 62
P = OH * OH  # 3844 patches per image
N_CORES = 8
IMGS_PER_CORE = B // N_CORES  # 2 (the two partition-chunks)
F = P
NPIX = HW * HW  # 4096
ROWCHUNK = 8  # patch rows per chunk (chunk = ROWCHUNK*62 <= 496 cols)

MM_DT = mybir.dt.float32r
F16 = mybir.dt.float16

# fp32 wmats: SGN8 (8 rows x 128 cols) + BIAS column
COL_SGN = 0
COL_BIAS = 128
WM_COLS = 129

# fp16 stage-matrix array: per kernel B0 (128) + W1..W7 (7*128) + W8 (64),
# then ZL2 (128 x 16) for the packed two-kernel measurement.
K16COLS = 128 + 7 * 128 + 64  # 1088
COL_ZL = 2 * K16COLS  # 2176
WM16_COLS = COL_ZL + 16  # 2192

# Per-stream multiply mode: 'A' = direct DVE fp32 (PSUM operand, 1x),
# 'B' = ScalarE copy PSUM->SBUF fp16 then DVE tensor_tensor at 2x,
# 'C' = ScalarE copy then GPSIMD tensor_tensor (SBUF-only engine).
C_STREAMS = {(1, 0), (4, 0), (7, 1)}
B_STREAMS = {(2, 0), (5, 0), (8, 0), (3, 1), (6, 1), (8, 1)}
WPCOLS = HW * OH  # 3968: patch-layout width (64 pixel rows x 62 cols)


def _mode(pos, k):
    if (pos, k) in C_STREAMS:
        return "C"
    if (pos, k) in B_STREAMS:
        return "B"
    return "A"

# ---------------- host-side constant math (weights-only, O(1)) ----------------
_H = np.array([[1, 1], [1, -1]], dtype=np.complex128) / np.sqrt(2)
_S = np.array([[1, 0], [0, 1j]], dtype=np.complex128)
_A1 = _S @ _H


def _kron_n(mats):
    out = np.array([[1.0 + 0j]])
    for m in mats:
        out = np.kron(out, m)
    return out


_AA = _kron_n([_A1] * IN_CH)
_U1 = _A1.conj().T @ (np.array([1.0, 1.0]) / np.sqrt(2))
_YINIT = _kron_n([_U1.reshape(2, 1)] * IN_CH).reshape(DIM)

_SGN = np.array(
    [[2 * ((i >> (3 - ch)) & 1) - 1 for i in range(DIM)] for ch in range(IN_CH)],
    dtype=np.float64,
)
_SIGMA = np.array(
    [[1 - 2 * ((i >> (3 - q)) & 1) for i in range(DIM)] for q in range(IN_CH)],
    dtype=np.float64,
)


def _rx(t):
    c, s = np.cos(t / 2), np.sin(t / 2)
    return np.array([[c, -1j * s], [-1j * s, c]])


def _ry(t):
    c, s = np.cos(t / 2), np.sin(t / 2)
    return np.array([[c, -s], [s, c]])


def _rz(t):
    e = np.exp(-0.5j * t)
    return np.array([[e, 0], [0, np.conj(e)]])


def _embed(U, q):
    mats = [np.eye(2, dtype=complex)] * IN_CH
    mats[q] = U
    return _kron_n(mats)


def _cx(cq, tq):
    M = np.zeros((DIM, DIM), dtype=complex)
    for i in range(DIM):
        bits = [(i >> (3 - q)) & 1 for q in range(4)]
        j = i
        if bits[cq] == 1:
            bits2 = bits.copy()
            bits2[tq] ^= 1
            j = sum(b << (3 - q) for q, b in enumerate(bits2))
        M[j, i] = 1
    return M


def _build_G(w_flat):
    w = np.float64(w_flat).reshape(NPOS, 1, IN_CH, 3)
    Gs = []
    for pos in range(NPOS):
        U = np.eye(DIM, dtype=complex)
        for q in range(IN_CH):
            R = _rz(w[pos, 0, q, 2]) @ _ry(w[pos, 0, q, 1]) @ _rx(w[pos, 0, q, 0])
            U = _embed(R, q) @ U
        for q in range(IN_CH - 1):
            U = _cx(q, q + 1) @ U
        U = _cx(IN_CH - 1, 0) @ U
        Gs.append(_AA.conj().T @ U @ _AA)
    G8f = _AA @ Gs[8]
    return Gs, G8f


def _wc_of(G, bouts):
    """Extended-state transition block: rows = m blocks [cR, cI, sI, sR],
    cols = output ext blocks listed in `bouts` (0/3 = Re, 1/2 = Im)."""
    Gr, Gi = G.real, G.imag
    Wc = np.zeros((64, 16 * len(bouts)))
    for o, bout in enumerate(bouts):
        re_out = bout in (0, 3)
        for j in range(DIM):
            col = o * 16 + j
            if re_out:
                Wc[0:16, col] = Gr[j]
                Wc[16:32, col] = -Gi[j]
                Wc[32:48, col] = -Gr[j]
                Wc[48:64, col] = -Gi[j]
            else:
                Wc[0:16, col] = Gi[j]
                Wc[16:32, col] = Gr[j]
                Wc[32:48, col] = -Gi[j]
                Wc[48:64, col] = Gr[j]
    return Wc


def _blockdiag2(M):
    Z = np.zeros((128, 2 * M.shape[1]), dtype=np.float32)
    Z[:64, : M.shape[1]] = M
    Z[64:, M.shape[1] :] = M
    return Z


# Triple-angle range reduction: the ScalarE Sin spline is only valid on
# [-pi, pi] but alpha reaches ~5. We compute v = sin(alpha/3 + delta)
# (delta = pi/6 for cos rows, 0 for sin rows), then w = (v^2 - 3/4) * v
# = -Phi/4, and absorb the -4 into the stage matrices.
_STAGE_SCALE = -4.0


def _build_wmats(weights):
    """(128, WM_COLS) fp32: SGN8 sign matrix + Sin bias column."""
    wm = np.zeros((128, WM_COLS), dtype=np.float32)
    # SGN8 (8, 128): row 4m+ch -> g rows 64m + 16b + i, entries _SGN[ch,i]/6
    # (emits alpha/3 directly).
    for m in range(IMGS_PER_CORE):
        for ch in range(IN_CH):
            for b in range(4):
                for i in range(DIM):
                    wm[4 * m + ch, COL_SGN + 64 * m + 16 * b + i] = _SGN[ch, i] / 6.0
    # BIAS (128, 1): pi/6 on cos rows (blocks 0,1), 0 on sin rows (blocks 2,3)
    bias = np.zeros(128, dtype=np.float32)
    for chunk in range(2):
        bias[chunk * 64 : chunk * 64 + 32] = np.pi / 6
    wm[:, COL_BIAS] = bias
    return wm


def _build_wmats16(weights):
    """(128, WM16_COLS) fp16: B0, W1..W7, W8 per kernel + ZL2."""
    wm = np.zeros((128, WM16_COLS), dtype=np.float16)
    for k in range(2):
        Gs, G8f = _build_G(weights[k])
        G0c = Gs[0] @ np.diag(_YINIT)
        # pos-0 matrix: rhs is the w-window itself ([c;c;s;s] blocks)
        B0c = np.zeros((64, 64))
        G0r, G0i = G0c.real, G0c.imag
        for bout in range(4):
            re_out = bout in (0, 3)
            for j in range(DIM):
                col = bout * 16 + j
                if re_out:
                    B0c[0:16, col] = G0r[j]
                    B0c[32:48, col] = -G0i[j]
                else:
                    B0c[0:16, col] = G0i[j]
                    B0c[32:48, col] = G0r[j]
        base = k * K16COLS
        wm[:, base : base + 128] = (_STAGE_SCALE * _blockdiag2(B0c)).astype(
            np.float16
        )
        for p in range(1, 8):
            wm[:, base + p * 128 : base + (p + 1) * 128] = (
                _STAGE_SCALE * _blockdiag2(_wc_of(Gs[p], [0, 1, 2, 3]))
            ).astype(np.float16)
        wm[:, base + 1024 : base + 1088] = (
            _STAGE_SCALE * _blockdiag2(_wc_of(G8f, [0, 1]))
        ).astype(np.float16)
    wm[:, COL_ZL : COL_ZL + 16] = _zl2().astype(np.float16)
    return wm


def _zl2():
    """ZL2 (128, 16) packed 2-kernel measurement matrix. Row
    64k + 32m + 16h + i, col 8k + 4m + q = SIGMA[q, i]."""
    zl = np.zeros((128, 16), dtype=np.float32)
    for k in range(2):
        for m in range(IMGS_PER_CORE):
            for h in range(2):
                for q in range(IN_CH):
                    for i in range(DIM):
                        zl[64 * k + 32 * m + 16 * h + i, 8 * k + 4 * m + q] = _SIGMA[
                            q, i
                        ]
    return zl


def _build_x2(x):
    """x (16, 4, 64, 64) -> per-core pixel arrays (8, 4096):
    rows = [img0 ch0..3, img1 ch0..3]."""
    xf = np.ascontiguousarray(x, dtype=np.float32).reshape(B, IN_CH, NPIX)
    return [
        np.ascontiguousarray(xf[2 * c : 2 * c + 2].reshape(8, NPIX))
        for c in range(N_CORES)
    ]


# ---------------- custom fused DVE op: w = ((v^2 - 3/4) * v) * y ------------
_CUBE_OP = None


def _register_cube_mul():
    """Register the fused triple-angle multiply as a custom DVE op."""
    global _CUBE_OP
    if _CUBE_OP is not None:
        return _CUBE_OP
    import concourse.dve_ops as dve_ops

    for o in dve_ops.OPS:
        if o.name == "CUBE_MUL_ANT":
            _CUBE_OP = o
            return o
    from concourse.dve_ops import DveOp
    from concourse.dve_spec import C0, Spec, Src0, Src1, lower
    from concourse.dve_uop import DveOpSpec

    body = ((Src0 * Src0 - C0) * Src0) * Src1
    spec = Spec(
        body=body,
        reference=lambda in0, in1, c0, c1, c2: (
            ((in0.astype(np.float32) * in0 - c0) * in0) * in1
        ),
    )
    row = max(dve_ops._SUB_OPCODE_FOR_NAME.values()) + 1
    shas = {}
    for ver in ("v3", "v4"):
        uops = lower(spec, ver=ver)
        shas[ver] = DveOpSpec(
            name="CUBE_MUL_ANT", opcode=row, uops=uops, rd1_en=True
        ).sha(ver)
    op = DveOp("CUBE_MUL_ANT", spec, subdim=False, uops_sha=shas)
    dve_ops.OPS.append(op)
    dve_ops._SUB_OPCODE_FOR_NAME[op.name] = row
    dve_ops.CUSTOM_DVE_SPECS[op.name] = spec
    _CUBE_OP = op
    return op


# ---------------- device program ----------------
_PROGRAM_CACHE = {}


def _build_program(mm_dt):
    key = str(mm_dt)
    if key in _PROGRAM_CACHE:
        return _PROGRAM_CACHE[key]

    nc = bacc.Bacc("TRN2", target_bir_lowering=False, debug=False)
    x2_d = nc.dram_tensor("xpix", [8, NPIX], mm_dt, kind="ExternalInput").ap()
    wm_d = nc.dram_tensor(
        "wmats", [128, WM_COLS], mm_dt, kind="ExternalInput"
    ).ap()
    wm16_d = nc.dram_tensor(
        "wmats16", [128, WM16_COLS], mybir.dt.float16, kind="ExternalInput"
    ).ap()
    z_d = nc.dram_tensor(
        "zout", [2, 8, F], mybir.dt.float32, kind="ExternalOutput"
    ).ap()

    f32 = mybir.dt.float32
    CUBE = _register_cube_mul()

    # patch-row chunks: (py0, nrows)
    chunks = []
    py0 = 0
    while py0 < OH:
        chunks.append((py0, min(ROWCHUNK, OH - py0)))
        py0 += ROWCHUNK
    nch = len(chunks)

    with tile.TileContext(nc) as tc:
        from contextlib import ExitStack

        with ExitStack() as ctx:
            const_pool = ctx.enter_context(tc.tile_pool(name="const", bufs=1))
            m16_pool = ctx.enter_context(tc.tile_pool(name="m16", bufs=8))
            y16_pool = ctx.enter_context(tc.tile_pool(name="y16", bufs=8))
            sq_pool = ctx.enter_context(tc.tile_pool(name="sq", bufs=2))
            zs_pool = ctx.enter_context(tc.tile_pool(name="zs", bufs=2))
            a_pool = ctx.enter_context(tc.tile_pool(name="aps", bufs=2, space="PSUM"))
            yp = [
                ctx.enter_context(tc.tile_pool(name=f"y{i}", bufs=2, space="PSUM"))
                for i in range(3)
            ]

            wm_sb = const_pool.tile([128, WM_COLS], mm_dt)
            nc.sync.dma_start(wm_sb[:], wm_d[:])
            wm16_sb = const_pool.tile([128, WM16_COLS], F16)
            nc.sync.dma_start(wm16_sb[:], wm16_d[:])
            x2_sb = const_pool.tile([8, NPIX], mm_dt)
            nc.sync.dma_start(x2_sb[:], x2_d[:])
            ones_sb = const_pool.tile([128, 512], f32)
            nc.vector.memset(ones_sb[:], 1.0)
            v_sb = const_pool.tile([128, NPIX], f32)
            w_sb = const_pool.tile([128, NPIX], mm_dt)
            wp16 = [
                const_pool.tile([128, WPCOLS], F16, name=f"wp16_{dx}")
                for dx in range(3)
            ]

            sgn_l = wm_sb[0:8, COL_SGN : COL_SGN + 128]
            z_l2 = wm16_sb[:, COL_ZL : COL_ZL + 16]
            bias_col = wm_sb[:, COL_BIAS : COL_BIAS + 1].bitcast(f32)

            def w16_ap(k, p):
                base = k * K16COLS
                if p == 0:
                    return wm16_sb[:, base : base + 128]
                if p < 8:
                    return wm16_sb[:, base + p * 128 : base + (p + 1) * 128]
                return wm16_sb[:, base + 1024 : base + 1088]

            def w_win(ci, pos):
                # fp32 shifted pixel-layout window (3 free dims)
                py0, nr = chunks[ci]
                dy, dx = divmod(pos, 3)
                ap = w_sb[:].rearrange("p (r c) -> p r c", c=HW)
                return ap[:, py0 + dy : py0 + dy + nr, dx : dx + OH]

            def wp_win(ci, pos):
                # fp16 contiguous patch-layout window
                py0, nr = chunks[ci]
                dy, dx = divmod(pos, 3)
                r0 = (py0 + dy) * OH
                return wp16[dx][:, r0 : r0 + nr * OH]

            # ---- prologue: g -> v -> w over the full pixel grid ----
            for j in range(NPIX // 512):
                g_ps = a_pool.tile([128, 512], f32, tag="aps", name=f"g{j}")
                nc.tensor.matmul(
                    g_ps[:],
                    sgn_l,
                    x2_sb[:, j * 512 : (j + 1) * 512],
                    start=True,
                    stop=True,
                )
                nc.scalar.activation(
                    v_sb[:, j * 512 : (j + 1) * 512],
                    g_ps[:],
                    mybir.ActivationFunctionType.Sin,
                    bias=bias_col,
                    scale=1.0,
                )
                nc.vector._custom_dve(
                    CUBE,
                    out=w_sb[:, j * 512 : (j + 1) * 512],
                    in0=v_sb[:, j * 512 : (j + 1) * 512],
                    in1=ones_sb[:],
                    s0=0.75,
                )
            # fp16 patch-layout pre-gathered copies of w (one per dx shift);
            # contiguous reads keep the 2x DVE mode clean.  Split into row
            # halves so early chunks unblock sooner.
            w_pix = w_sb[:].rearrange("p (r c) -> p r c", c=HW)
            for dx in range(3):
                for h in range(2):
                    nc.vector.tensor_copy(
                        wp16[dx][:, h * 32 * OH : (h + 1) * 32 * OH],
                        w_pix[:, h * 32 : (h + 1) * 32, dx : dx + OH].bitcast(f32),
                    )

            chain = {}

            def do_state(ci, pos):
                py0, nr = chunks[ci]
                C = nr * OH
                cur = chain.setdefault(ci, [None, None])
                # pos 8: both kernels' final [Re;Im] states share one tile
                # (k0 -> rows 0:64, k1 -> rows 64:128) for a packed measure.
                y8 = None
                if pos == 8:
                    y8 = yp[(2 * ci) % 3].tile(
                        [128, 512], f32, tag=f"y{(2 * ci) % 3}", name=f"y8_{ci}"
                    )
                for k in range(2):
                    if pos == 8:
                        y_new = y8
                        out_ap = y8[64 * k : 64 * k + 64, :C]
                    else:
                        pool = yp[(2 * ci + k) % 3]
                        y_new = pool.tile(
                            [128, 512],
                            f32,
                            tag=f"y{(2 * ci + k) % 3}",
                            name=f"y{ci}_{k}_{pos}",
                        )
                        out_ap = y_new[:, :C]
                    if pos == 0:
                        nc.tensor.matmul(
                            out_ap, w16_ap(k, 0), wp_win(ci, 0),
                            start=True, stop=True,
                        )
                    else:
                        mode = _mode(pos, k)
                        m16 = m16_pool.tile([128, 512], F16, tag="m16")
                        if mode == "A":
                            nc.vector.tensor_mul(
                                m16[:, :C], w_win(ci, pos), cur[k][:, :C]
                            )
                        else:
                            y16 = y16_pool.tile([128, 512], F16, tag="y16")
                            nc.scalar.copy(y16[:, :C], cur[k][:, :C])
                            eng = nc.vector if mode == "B" else nc.gpsimd
                            eng.tensor_mul(
                                m16[:, :C], wp_win(ci, pos), y16[:, :C]
                            )
                        nc.tensor.matmul(
                            out_ap, w16_ap(k, pos), m16[:, :C],
                            start=True, stop=True,
                        )
                    cur[k] = y_new

            def do_meas(ci, _pos):
                py0, nr = chunks[ci]
                C = nr * OH
                c0 = py0 * OH
                y8 = chain.pop(ci)[0]
                sq = sq_pool.tile([128, 512], F16, tag="sq")
                nc.scalar.activation(
                    sq[:, :C], y8[:, :C], mybir.ActivationFunctionType.Square
                )
                zq = a_pool.tile([128, 512], f32, tag="aps", name=f"zq{ci}")
                nc.tensor.matmul(zq[0:16, :C], z_l2, sq[:, :C], start=True, stop=True)
                zs = zs_pool.tile([16, 512], f32, tag="zs")
                nc.scalar.copy(zs[0:16, :C], zq[0:16, :C])
                nc.sync.dma_start(
                    z_d[:, :, c0 : c0 + C].rearrange("a b c -> (a b) c"),
                    zs[0:16, :C],
                )

            # merged time-ordered emission: chains pipelined across chunks
            OFF = 2.0
            events = []
            for ci in range(nch):
                t0 = ci * OFF
                for pos in range(NPOS):
                    events.append((t0 + pos, 2, ci, pos, do_state))
                events.append((t0 + NPOS, 3, ci, 0, do_meas))
            events.sort(key=lambda e: (e[0], e[1], e[2]))
            for _t, _kind, ci, pos, fn in events:
                fn(ci, pos)

    nc.compile()
    _PROGRAM_CACHE[key] = nc
    return nc


# ---------------- entry point ----------------
def kernel(x, weights):
    x = np.asarray(x, dtype=np.float32)
    weights = np.asarray(weights, dtype=np.float32)
    wm = _build_wmats(weights)
    wm16 = _build_wmats16(weights)
    x2s = _build_x2(x)

    nc = _build_program(MM_DT)
    in_maps = [
        {"xpix": x2s[c], "wmats": wm, "wmats16": wm16} for c in range(N_CORES)
    ]
    res = run_bass_kernel_spmd(nc, in_maps, list(range(N_CORES)))

    out = np.zeros((B, 2 * IN_CH, OH, OH), dtype=np.float32)
    for c in range(N_CORES):
        z = np.asarray(res.results[c]["zout"])  # (2, 8, F)
        for k in range(2):
            for chunk in range(2):
                b = 2 * c + chunk
                out[b, k * 4 : k * 4 + 4] = z[k, chunk * 4 : chunk * 4 + 4].reshape(
                    IN_CH, OH, OH
                )
    return out


# revision 30
# speedup vs baseline: 1.0385x; 1.0059x over previous
"""Trainium2 Bass kernel for nn_ChannelWiseQuantumKernel.

Method: the per-position data RY gates are converted to diagonal phase gates
via RY(t) = (SH) RZ(t) (SH)^dag, so the circuit per patch becomes
    y <- G_pos (e^{i a} . y),   a_i = 0.5 * sum_ch (2 b_ch(i) - 1) theta_ch
with fixed 16x16 complex G_pos precomputed on host from the (tiny) weights.

Key structural trick ("g-trick"): patches overlap, so the phase angles for
position (dy,dx) at patch (py,px) are functions of PIXEL (py+dy, px+dx) only.
We compute a per-pixel sign-combination tensor g (128 rows x 4096 pixels) with
ONE small matmul per core, one Sin activation pass, and one cube
(triple-angle range reduction) pass; every position then reads the resulting
phase tensor w via a shifted 2-D window access pattern.  Per position the
device does only: ONE VectorE multiply (w-window x state, extended layout
[Re;Im;Im;Re] x [c;c;s;s] giving all 4 real products) and ONE 128x128
tensor-engine matmul.  Position 0 needs no multiply at all (the matmul's
moving operand is the w-window itself; the initial state is folded into B0).
State lives in PSUM ping-pong banks; patches stream in patch-row chunks.
Measurement: Square (ScalarE) + one matmul against a signed-sum matrix.
"""

import sys

sys.path.insert(0, "/opt/trn_rl_repo")

import numpy as np

import concourse.bacc as bacc
import concourse.bass as bass
import concourse.tile as tile
from concourse import mybir
from concourse.bass_utils import run_bass_kernel_spmd

# ---------------- problem constants ----------------
IN_CH = 4
KSZ = 3
NPOS = 9
DIM = 16
B = 16
HW = 64
OH = HW - KSZ + 1  # 62
P = OH * OH  # 3844 patches per image
N_CORES = 8
IMGS_PER_CORE = B // N_CORES  # 2 (the two partition-chunks)
F = P
NPIX = HW * HW  # 4096
ROWCHUNK = 8  # patch rows per chunk (chunk = ROWCHUNK*62 <= 496 cols)

MM_DT = mybir.dt.float32r
F16 = mybir.dt.float16

# fp32 wmats: SGN8 (8 rows x 128 cols) + BIAS column
COL_SGN = 0
COL_BIAS = 128
WM_COLS = 129

# fp16 stage-matrix array: per kernel B0 (128) + W1..W7 (7*128) + W8 (64),
# then ZL2 (128 x 16) for the packed two-kernel measurement.
K16COLS = 128 + 7 * 128 + 64  # 1088
COL_ZL = 2 * K16COLS  # 2176
WM16_COLS = COL_ZL + 16  # 2192

# Per-stream multiply mode: 'A' = direct DVE fp32 (PSUM operand, 1x),
# 'B' = ScalarE copy PSUM->SBUF fp16 then DVE tensor_tensor at 2x,
# 'C' = ScalarE copy then GPSIMD tensor_tensor (SBUF-only engine).
C_STREAMS = set()
B_STREAMS = {(2, 0), (5, 0), (8, 0), (3, 1), (6, 1), (8, 1)}
WPCOLS = HW * OH  # 3968: patch-layout width (64 pixel rows x 62 cols)


def _mode(pos, k):
    if (pos, k) in C_STREAMS:
        return "C"
    if (pos, k) in B_STREAMS:
        return "B"
    return "A"

# ---------------- host-side constant math (weights-only, O(1)) ----------------
_H = np.array([[1, 1], [1, -1]], dtype=np.complex128) / np.sqrt(2)
_S = np.array([[1, 0], [0, 1j]], dtype=np.complex128)
_A1 = _S @ _H


def _kron_n(mats):
    out = np.array([[1.0 + 0j]])
    for m in mats:
        out = np.kron(out, m)
    return out


_AA = _kron_n([_A1] * IN_CH)
_U1 = _A1.conj().T @ (np.array([1.0, 1.0]) / np.sqrt(2))
_YINIT = _kron_n([_U1.reshape(2, 1)] * IN_CH).reshape(DIM)

_SGN = np.array(
    [[2 * ((i >> (3 - ch)) & 1) - 1 for i in range(DIM)] for ch in range(IN_CH)],
    dtype=np.float64,
)
_SIGMA = np.array(
    [[1 - 2 * ((i >> (3 - q)) & 1) for i in range(DIM)] for q in range(IN_CH)],
    dtype=np.float64,
)


def _rx(t):
    c, s = np.cos(t / 2), np.sin(t / 2)
    return np.array([[c, -1j * s], [-1j * s, c]])


def _ry(t):
    c, s = np.cos(t / 2), np.sin(t / 2)
    return np.array([[c, -s], [s, c]])


def _rz(t):
    e = np.exp(-0.5j * t)
    return np.array([[e, 0], [0, np.conj(e)]])


def _embed(U, q):
    mats = [np.eye(2, dtype=complex)] * IN_CH
    mats[q] = U
    return _kron_n(mats)


def _cx(cq, tq):
    M = np.zeros((DIM, DIM), dtype=complex)
    for i in range(DIM):
        bits = [(i >> (3 - q)) & 1 for q in range(4)]
        j = i
        if bits[cq] == 1:
            bits2 = bits.copy()
            bits2[tq] ^= 1
            j = sum(b << (3 - q) for q, b in enumerate(bits2))
        M[j, i] = 1
    return M


def _build_G(w_flat):
    w = np.float64(w_flat).reshape(NPOS, 1, IN_CH, 3)
    Gs = []
    for pos in range(NPOS):
        U = np.eye(DIM, dtype=complex)
        for q in range(IN_CH):
            R = _rz(w[pos, 0, q, 2]) @ _ry(w[pos, 0, q, 1]) @ _rx(w[pos, 0, q, 0])
            U = _embed(R, q) @ U
        for q in range(IN_CH - 1):
            U = _cx(q, q + 1) @ U
        U = _cx(IN_CH - 1, 0) @ U
        Gs.append(_AA.conj().T @ U @ _AA)
    G8f = _AA @ Gs[8]
    return Gs, G8f


def _wc_of(G, bouts):
    """Extended-state transition block: rows = m blocks [cR, cI, sI, sR],
    cols = output ext blocks listed in `bouts` (0/3 = Re, 1/2 = Im)."""
    Gr, Gi = G.real, G.imag
    Wc = np.zeros((64, 16 * len(bouts)))
    for o, bout in enumerate(bouts):
        re_out = bout in (0, 3)
        for j in range(DIM):
            col = o * 16 + j
            if re_out:
                Wc[0:16, col] = Gr[j]
                Wc[16:32, col] = -Gi[j]
                Wc[32:48, col] = -Gr[j]
                Wc[48:64, col] = -Gi[j]
            else:
                Wc[0:16, col] = Gi[j]
                Wc[16:32, col] = Gr[j]
                Wc[32:48, col] = -Gi[j]
                Wc[48:64, col] = Gr[j]
    return Wc


def _blockdiag2(M):
    Z = np.zeros((128, 2 * M.shape[1]), dtype=np.float32)
    Z[:64, : M.shape[1]] = M
    Z[64:, M.shape[1] :] = M
    return Z


# Triple-angle range reduction: the ScalarE Sin spline is only valid on
# [-pi, pi] but alpha reaches ~5. We compute v = sin(alpha/3 + delta)
# (delta = pi/6 for cos rows, 0 for sin rows), then w = (v^2 - 3/4) * v
# = -Phi/4, and absorb the -4 into the stage matrices.
_STAGE_SCALE = -4.0


def _build_wmats(weights):
    """(128, WM_COLS) fp32: SGN8 sign matrix + Sin bias column."""
    wm = np.zeros((128, WM_COLS), dtype=np.float32)
    # SGN8 (8, 128): row 4m+ch -> g rows 64m + 16b + i, entries _SGN[ch,i]/6
    # (emits alpha/3 directly).
    for m in range(IMGS_PER_CORE):
        for ch in range(IN_CH):
            for b in range(4):
                for i in range(DIM):
                    wm[4 * m + ch, COL_SGN + 64 * m + 16 * b + i] = _SGN[ch, i] / 6.0
    # BIAS (128, 1): pi/6 on cos rows (blocks 0,1), 0 on sin rows (blocks 2,3)
    bias = np.zeros(128, dtype=np.float32)
    for chunk in range(2):
        bias[chunk * 64 : chunk * 64 + 32] = np.pi / 6
    wm[:, COL_BIAS] = bias
    return wm


def _build_wmats16(weights):
    """(128, WM16_COLS) fp16: B0, W1..W7, W8 per kernel + ZL2."""
    wm = np.zeros((128, WM16_COLS), dtype=np.float16)
    for k in range(2):
        Gs, G8f = _build_G(weights[k])
        G0c = Gs[0] @ np.diag(_YINIT)
        # pos-0 matrix: rhs is the w-window itself ([c;c;s;s] blocks)
        B0c = np.zeros((64, 64))
        G0r, G0i = G0c.real, G0c.imag
        for bout in range(4):
            re_out = bout in (0, 3)
            for j in range(DIM):
                col = bout * 16 + j
                if re_out:
                    B0c[0:16, col] = G0r[j]
                    B0c[32:48, col] = -G0i[j]
                else:
                    B0c[0:16, col] = G0i[j]
                    B0c[32:48, col] = G0r[j]
        base = k * K16COLS
        wm[:, base : base + 128] = (_STAGE_SCALE * _blockdiag2(B0c)).astype(
            np.float16
        )
        for p in range(1, 8):
            wm[:, base + p * 128 : base + (p + 1) * 128] = (
                _STAGE_SCALE * _blockdiag2(_wc_of(Gs[p], [0, 1, 2, 3]))
            ).astype(np.float16)
        wm[:, base + 1024 : base + 1088] = (
            _STAGE_SCALE * _blockdiag2(_wc_of(G8f, [0, 1]))
        ).astype(np.float16)
    wm[:, COL_ZL : COL_ZL + 16] = _zl2().astype(np.float16)
    return wm


def _zl2():
    """ZL2 (128, 16) packed 2-kernel measurement matrix. Row
    64k + 32m + 16h + i, col 8k + 4m + q = SIGMA[q, i]."""
    zl = np.zeros((128, 16), dtype=np.float32)
    for k in range(2):
        for m in range(IMGS_PER_CORE):
            for h in range(2):
                for q in range(IN_CH):
                    for i in range(DIM):
                        zl[64 * k + 32 * m + 16 * h + i, 8 * k + 4 * m + q] = _SIGMA[
                            q, i
                        ]
    return zl


def _build_x2(x):
    """x (16, 4, 64, 64) -> per-core pixel arrays (8, 4096):
    rows = [img0 ch0..3, img1 ch0..3]."""
    xf = np.ascontiguousarray(x, dtype=np.float32).reshape(B, IN_CH, NPIX)
    return [
        np.ascontiguousarray(xf[2 * c : 2 * c + 2].reshape(8, NPIX))
        for c in range(N_CORES)
    ]


# ---------------- custom fused DVE op: w = ((v^2 - 3/4) * v) * y ------------
_CUBE_OP = None


def _register_cube_mul():
    """Register the fused triple-angle multiply as a custom DVE op."""
    global _CUBE_OP
    if _CUBE_OP is not None:
        return _CUBE_OP
    import concourse.dve_ops as dve_ops

    for o in dve_ops.OPS:
        if o.name == "CUBE_MUL_ANT":
            _CUBE_OP = o
            return o
    from concourse.dve_ops import DveOp
    from concourse.dve_spec import C0, Spec, Src0, Src1, lower
    from concourse.dve_uop import DveOpSpec

    body = ((Src0 * Src0 - C0) * Src0) * Src1
    spec = Spec(
        body=body,
        reference=lambda in0, in1, c0, c1, c2: (
            ((in0.astype(np.float32) * in0 - c0) * in0) * in1
        ),
    )
    row = max(dve_ops._SUB_OPCODE_FOR_NAME.values()) + 1
    shas = {}
    for ver in ("v3", "v4"):
        uops = lower(spec, ver=ver)
        shas[ver] = DveOpSpec(
            name="CUBE_MUL_ANT", opcode=row, uops=uops, rd1_en=True
        ).sha(ver)
    op = DveOp("CUBE_MUL_ANT", spec, subdim=False, uops_sha=shas)
    dve_ops.OPS.append(op)
    dve_ops._SUB_OPCODE_FOR_NAME[op.name] = row
    dve_ops.CUSTOM_DVE_SPECS[op.name] = spec
    _CUBE_OP = op
    return op


# ---------------- device program ----------------
_PROGRAM_CACHE = {}


def _build_program(mm_dt):
    key = str(mm_dt)
    if key in _PROGRAM_CACHE:
        return _PROGRAM_CACHE[key]

    nc = bacc.Bacc("TRN2", target_bir_lowering=False, debug=False)
    x2_d = nc.dram_tensor("xpix", [8, NPIX], mm_dt, kind="ExternalInput").ap()
    wm_d = nc.dram_tensor(
        "wmats", [128, WM_COLS], mm_dt, kind="ExternalInput"
    ).ap()
    wm16_d = nc.dram_tensor(
        "wmats16", [128, WM16_COLS], mybir.dt.float16, kind="ExternalInput"
    ).ap()
    z_d = nc.dram_tensor(
        "zout", [2, 8, F], mybir.dt.float32, kind="ExternalOutput"
    ).ap()

    f32 = mybir.dt.float32
    CUBE = _register_cube_mul()

    # patch-row chunks: (py0, nrows)
    chunks = []
    py0 = 0
    while py0 < OH:
        chunks.append((py0, min(ROWCHUNK, OH - py0)))
        py0 += ROWCHUNK
    nch = len(chunks)

    with tile.TileContext(nc) as tc:
        from contextlib import ExitStack

        with ExitStack() as ctx:
            const_pool = ctx.enter_context(tc.tile_pool(name="const", bufs=1))
            m16_pool = ctx.enter_context(tc.tile_pool(name="m16", bufs=8))
            y16_pool = ctx.enter_context(tc.tile_pool(name="y16", bufs=8))
            sq_pool = ctx.enter_context(tc.tile_pool(name="sq", bufs=2))
            zs_pool = ctx.enter_context(tc.tile_pool(name="zs", bufs=2))
            a_pool = ctx.enter_context(tc.tile_pool(name="aps", bufs=2, space="PSUM"))
            yp = [
                ctx.enter_context(tc.tile_pool(name=f"y{i}", bufs=2, space="PSUM"))
                for i in range(3)
            ]

            wm_sb = const_pool.tile([128, WM_COLS], mm_dt)
            nc.sync.dma_start(wm_sb[:], wm_d[:])
            wm16_sb = const_pool.tile([128, WM16_COLS], F16)
            nc.sync.dma_start(wm16_sb[:], wm16_d[:])
            x2_sb = const_pool.tile([8, NPIX], mm_dt)
            nc.sync.dma_start(x2_sb[:], x2_d[:])
            ones_sb = const_pool.tile([128, 512], f32)
            nc.vector.memset(ones_sb[:], 1.0)
            v_sb = const_pool.tile([128, NPIX], f32)
            w_sb = const_pool.tile([128, NPIX], mm_dt)
            wp16 = [
                const_pool.tile([128, WPCOLS], F16, name=f"wp16_{dx}")
                for dx in range(3)
            ]

            sgn_l = wm_sb[0:8, COL_SGN : COL_SGN + 128]
            z_l2 = wm16_sb[:, COL_ZL : COL_ZL + 16]
            bias_col = wm_sb[:, COL_BIAS : COL_BIAS + 1].bitcast(f32)

            def w16_ap(k, p):
                base = k * K16COLS
                if p == 0:
                    return wm16_sb[:, base : base + 128]
                if p < 8:
                    return wm16_sb[:, base + p * 128 : base + (p + 1) * 128]
                return wm16_sb[:, base + 1024 : base + 1088]

            def w_win(ci, pos):
                # fp32 shifted pixel-layout window (3 free dims)
                py0, nr = chunks[ci]
                dy, dx = divmod(pos, 3)
                ap = w_sb[:].rearrange("p (r c) -> p r c", c=HW)
                return ap[:, py0 + dy : py0 + dy + nr, dx : dx + OH]

            def wp_win(ci, pos):
                # fp16 contiguous patch-layout window
                py0, nr = chunks[ci]
                dy, dx = divmod(pos, 3)
                r0 = (py0 + dy) * OH
                return wp16[dx][:, r0 : r0 + nr * OH]

            # ---- prologue: g -> v -> w over the full pixel grid ----
            for j in range(NPIX // 512):
                g_ps = a_pool.tile([128, 512], f32, tag="aps", name=f"g{j}")
                nc.tensor.matmul(
                    g_ps[:],
                    sgn_l,
                    x2_sb[:, j * 512 : (j + 1) * 512],
                    start=True,
                    stop=True,
                )
                nc.scalar.activation(
                    v_sb[:, j * 512 : (j + 1) * 512],
                    g_ps[:],
                    mybir.ActivationFunctionType.Sin,
                    bias=bias_col,
                    scale=1.0,
                )
                nc.vector._custom_dve(
                    CUBE,
                    out=w_sb[:, j * 512 : (j + 1) * 512],
                    in0=v_sb[:, j * 512 : (j + 1) * 512],
                    in1=ones_sb[:],
                    s0=0.75,
                )
            # fp16 patch-layout pre-gathered copies of w (one per dx shift);
            # contiguous reads keep the 2x DVE mode clean.  Split into row
            # halves so early chunks unblock sooner.
            w_pix = w_sb[:].rearrange("p (r c) -> p r c", c=HW)
            for dx in range(3):
                for h in range(2):
                    nc.vector.tensor_copy(
                        wp16[dx][:, h * 32 * OH : (h + 1) * 32 * OH],
                        w_pix[:, h * 32 : (h + 1) * 32, dx : dx + OH].bitcast(f32),
                    )

            # PE warm-up: a back-to-back burst of dummy matmuls flips the
            # HAM clock gate to 8/8 (2.4 GHz) before the state chains start.
            wu = a_pool.tile([128, 512], f32, tag="aps", name="wu")
            for wi in range(12):
                nc.tensor.matmul(
                    wu[:, 0:128],
                    ones_sb[:, 0:128],
                    ones_sb[:, 0:128],
                    start=True,
                    stop=True,
                )

            chain = {}

            def do_state(ci, pos):
                py0, nr = chunks[ci]
                C = nr * OH
                cur = chain.setdefault(ci, [None, None])
                # pos 8: both kernels' final [Re;Im] states share one tile
                # (k0 -> rows 0:64, k1 -> rows 64:128) for a packed measure.
                y8 = None
                if pos == 8:
                    y8 = yp[(2 * ci) % 3].tile(
                        [128, 512], f32, tag=f"y{(2 * ci) % 3}", name=f"y8_{ci}"
                    )
                for k in range(2):
                    if pos == 8:
                        y_new = y8
                        out_ap = y8[64 * k : 64 * k + 64, :C]
                    else:
                        pool = yp[(2 * ci + k) % 3]
                        y_new = pool.tile(
                            [128, 512],
                            f32,
                            tag=f"y{(2 * ci + k) % 3}",
                            name=f"y{ci}_{k}_{pos}",
                        )
                        out_ap = y_new[:, :C]
                    if pos == 0:
                        nc.tensor.matmul(
                            out_ap, w16_ap(k, 0), wp_win(ci, 0),
                            start=True, stop=True,
                        )
                    else:
                        mode = _mode(pos, k)
                        m16 = m16_pool.tile([128, 512], F16, tag="m16")
                        if mode == "A":
                            nc.vector.tensor_mul(
                                m16[:, :C], w_win(ci, pos), cur[k][:, :C]
                            )
                        else:
                            y16 = y16_pool.tile([128, 512], F16, tag="y16")
                            nc.scalar.copy(y16[:, :C], cur[k][:, :C])
                            eng = nc.vector if mode == "B" else nc.gpsimd
                            eng.tensor_mul(
                                m16[:, :C], wp_win(ci, pos), y16[:, :C]
                            )
                        nc.tensor.matmul(
                            out_ap, w16_ap(k, pos), m16[:, :C],
                            start=True, stop=True,
                        )
                    cur[k] = y_new

            def do_meas(ci, _pos):
                py0, nr = chunks[ci]
                C = nr * OH
                c0 = py0 * OH
                y8 = chain.pop(ci)[0]
                sq = sq_pool.tile([128, 512], F16, tag="sq")
                nc.scalar.activation(
                    sq[:, :C], y8[:, :C], mybir.ActivationFunctionType.Square
                )
                zq = a_pool.tile([128, 512], f32, tag="aps", name=f"zq{ci}")
                nc.tensor.matmul(zq[0:16, :C], z_l2, sq[:, :C], start=True, stop=True)
                zs = zs_pool.tile([16, 512], f32, tag="zs")
                nc.scalar.copy(zs[0:16, :C], zq[0:16, :C])
                nc.sync.dma_start(
                    z_d[:, :, c0 : c0 + C].rearrange("a b c -> (a b) c"),
                    zs[0:16, :C],
                )

            # merged time-ordered emission: chains pipelined across chunks
            OFF = 2.0
            events = []
            for ci in range(nch):
                t0 = ci * OFF
                for pos in range(NPOS):
                    events.append((t0 + pos, 2, ci, pos, do_state))
                events.append((t0 + NPOS, 3, ci, 0, do_meas))
            events.sort(key=lambda e: (e[0], e[1], e[2]))
            for _t, _kind, ci, pos, fn in events:
                fn(ci, pos)

    nc.compile()
    _PROGRAM_CACHE[key] = nc
    return nc


# ---------------- entry point ----------------
def kernel(x, weights):
    x = np.asarray(x, dtype=np.float32)
    weights = np.asarray(weights, dtype=np.float32)
    wm = _build_wmats(weights)
    wm16 = _build_wmats16(weights)
    x2s = _build_x2(x)

    nc = _build_program(MM_DT)
    in_maps = [
        {"xpix": x2s[c], "wmats": wm, "wmats16": wm16} for c in range(N_CORES)
    ]
    res = run_bass_kernel_spmd(nc, in_maps, list(range(N_CORES)))

    out = np.zeros((B, 2 * IN_CH, OH, OH), dtype=np.float32)
    for c in range(N_CORES):
        z = np.asarray(res.results[c]["zout"])  # (2, 8, F)
        for k in range(2):
            for chunk in range(2):
                b = 2 * c + chunk
                out[b, k * 4 : k * 4 + 4] = z[k, chunk * 4 : chunk * 4 + 4].reshape(
                    IN_CH, OH, OH
                )
    return out


# revision 32
# speedup vs baseline: 1.0437x; 1.0049x over previous
"""Trainium2 Bass kernel for nn_ChannelWiseQuantumKernel.

Method: the per-position data RY gates are converted to diagonal phase gates
via RY(t) = (SH) RZ(t) (SH)^dag, so the circuit per patch becomes
    y <- G_pos (e^{i a} . y),   a_i = 0.5 * sum_ch (2 b_ch(i) - 1) theta_ch
with fixed 16x16 complex G_pos precomputed on host from the (tiny) weights.

Key structural trick ("g-trick"): patches overlap, so the phase angles for
position (dy,dx) at patch (py,px) are functions of PIXEL (py+dy, px+dx) only.
We compute a per-pixel sign-combination tensor g (128 rows x 4096 pixels) with
ONE small matmul per core, one Sin activation pass, and one cube
(triple-angle range reduction) pass; every position then reads the resulting
phase tensor w via a shifted 2-D window access pattern.  Per position the
device does only: ONE VectorE multiply (w-window x state, extended layout
[Re;Im;Im;Re] x [c;c;s;s] giving all 4 real products) and ONE 128x128
tensor-engine matmul.  Position 0 needs no multiply at all (the matmul's
moving operand is the w-window itself; the initial state is folded into B0).
State lives in PSUM ping-pong banks; patches stream in patch-row chunks.
Measurement: Square (ScalarE) + one matmul against a signed-sum matrix.
"""

import sys

sys.path.insert(0, "/opt/trn_rl_repo")

import numpy as np

import concourse.bacc as bacc
import concourse.bass as bass
import concourse.tile as tile
from concourse import mybir
from concourse.bass_utils import run_bass_kernel_spmd

# ---------------- problem constants ----------------
IN_CH = 4
KSZ = 3
NPOS = 9
DIM = 16
B = 16
HW = 64
OH = HW - KSZ + 1  # 62
P = OH * OH  # 3844 patches per image
N_CORES = 8
IMGS_PER_CORE = B // N_CORES  # 2 (the two partition-chunks)
F = P
NPIX = HW * HW  # 4096
ROWCHUNK = 8  # patch rows per chunk (chunk = ROWCHUNK*62 <= 496 cols)

MM_DT = mybir.dt.float32r
F16 = mybir.dt.float16

# fp32 wmats: SGN8 (8 rows x 128 cols) + BIAS column
COL_SGN = 0
COL_BIAS = 128
WM_COLS = 129

# fp16 stage-matrix array: per kernel B0 (128) + W1..W7 (7*128) + W8 (64),
# then ZL2 (128 x 16) for the packed two-kernel measurement.
K16COLS = 128 + 7 * 128 + 64  # 1088
COL_ZL = 2 * K16COLS  # 2176
WM16_COLS = COL_ZL + 16  # 2192

# Per-stream multiply mode: 'A' = direct DVE fp32 (PSUM operand, 1x),
# 'B' = ScalarE copy PSUM->SBUF fp16 then DVE tensor_tensor at 2x,
# 'C' = ScalarE copy then GPSIMD tensor_tensor (SBUF-only engine).
C_STREAMS = set()
B_STREAMS = {(2, 0), (5, 0), (8, 0), (3, 1), (6, 1), (8, 1)}
WPCOLS = HW * OH  # 3968: patch-layout width (64 pixel rows x 62 cols)


def _mode(pos, k):
    if (pos, k) in C_STREAMS:
        return "C"
    if (pos, k) in B_STREAMS:
        return "B"
    return "A"

# ---------------- host-side constant math (weights-only, O(1)) ----------------
_H = np.array([[1, 1], [1, -1]], dtype=np.complex128) / np.sqrt(2)
_S = np.array([[1, 0], [0, 1j]], dtype=np.complex128)
_A1 = _S @ _H


def _kron_n(mats):
    out = np.array([[1.0 + 0j]])
    for m in mats:
        out = np.kron(out, m)
    return out


_AA = _kron_n([_A1] * IN_CH)
_U1 = _A1.conj().T @ (np.array([1.0, 1.0]) / np.sqrt(2))
_YINIT = _kron_n([_U1.reshape(2, 1)] * IN_CH).reshape(DIM)

_SGN = np.array(
    [[2 * ((i >> (3 - ch)) & 1) - 1 for i in range(DIM)] for ch in range(IN_CH)],
    dtype=np.float64,
)
_SIGMA = np.array(
    [[1 - 2 * ((i >> (3 - q)) & 1) for i in range(DIM)] for q in range(IN_CH)],
    dtype=np.float64,
)


def _rx(t):
    c, s = np.cos(t / 2), np.sin(t / 2)
    return np.array([[c, -1j * s], [-1j * s, c]])


def _ry(t):
    c, s = np.cos(t / 2), np.sin(t / 2)
    return np.array([[c, -s], [s, c]])


def _rz(t):
    e = np.exp(-0.5j * t)
    return np.array([[e, 0], [0, np.conj(e)]])


def _embed(U, q):
    mats = [np.eye(2, dtype=complex)] * IN_CH
    mats[q] = U
    return _kron_n(mats)


def _cx(cq, tq):
    M = np.zeros((DIM, DIM), dtype=complex)
    for i in range(DIM):
        bits = [(i >> (3 - q)) & 1 for q in range(4)]
        j = i
        if bits[cq] == 1:
            bits2 = bits.copy()
            bits2[tq] ^= 1
            j = sum(b << (3 - q) for q, b in enumerate(bits2))
        M[j, i] = 1
    return M


def _build_G(w_flat):
    w = np.float64(w_flat).reshape(NPOS, 1, IN_CH, 3)
    Gs = []
    for pos in range(NPOS):
        U = np.eye(DIM, dtype=complex)
        for q in range(IN_CH):
            R = _rz(w[pos, 0, q, 2]) @ _ry(w[pos, 0, q, 1]) @ _rx(w[pos, 0, q, 0])
            U = _embed(R, q) @ U
        for q in range(IN_CH - 1):
            U = _cx(q, q + 1) @ U
        U = _cx(IN_CH - 1, 0) @ U
        Gs.append(_AA.conj().T @ U @ _AA)
    G8f = _AA @ Gs[8]
    return Gs, G8f


def _wc_of(G, bouts):
    """Extended-state transition block: rows = m blocks [cR, cI, sI, sR],
    cols = output ext blocks listed in `bouts` (0/3 = Re, 1/2 = Im)."""
    Gr, Gi = G.real, G.imag
    Wc = np.zeros((64, 16 * len(bouts)))
    for o, bout in enumerate(bouts):
        re_out = bout in (0, 3)
        for j in range(DIM):
            col = o * 16 + j
            if re_out:
                Wc[0:16, col] = Gr[j]
                Wc[16:32, col] = -Gi[j]
                Wc[32:48, col] = -Gr[j]
                Wc[48:64, col] = -Gi[j]
            else:
                Wc[0:16, col] = Gi[j]
                Wc[16:32, col] = Gr[j]
                Wc[32:48, col] = -Gi[j]
                Wc[48:64, col] = Gr[j]
    return Wc


def _blockdiag2(M):
    Z = np.zeros((128, 2 * M.shape[1]), dtype=np.float32)
    Z[:64, : M.shape[1]] = M
    Z[64:, M.shape[1] :] = M
    return Z


# Triple-angle range reduction: the ScalarE Sin spline is only valid on
# [-pi, pi] but alpha reaches ~5. We compute v = sin(alpha/3 + delta)
# (delta = pi/6 for cos rows, 0 for sin rows), then w = (v^2 - 3/4) * v
# = -Phi/4, and absorb the -4 into the stage matrices.
_STAGE_SCALE = -4.0


def _build_wmats(weights):
    """(128, WM_COLS) fp32: SGN8 sign matrix + Sin bias column."""
    wm = np.zeros((128, WM_COLS), dtype=np.float32)
    # SGN8 (8, 128): row 4m+ch -> g rows 64m + 16b + i, entries _SGN[ch,i]/6
    # (emits alpha/3 directly).
    for m in range(IMGS_PER_CORE):
        for ch in range(IN_CH):
            for b in range(4):
                for i in range(DIM):
                    wm[4 * m + ch, COL_SGN + 64 * m + 16 * b + i] = _SGN[ch, i] / 6.0
    # BIAS (128, 1): pi/6 on cos rows (blocks 0,1), 0 on sin rows (blocks 2,3)
    bias = np.zeros(128, dtype=np.float32)
    for chunk in range(2):
        bias[chunk * 64 : chunk * 64 + 32] = np.pi / 6
    wm[:, COL_BIAS] = bias
    return wm


def _build_wmats16(weights):
    """(128, WM16_COLS) fp16: B0, W1..W7, W8 per kernel + ZL2."""
    wm = np.zeros((128, WM16_COLS), dtype=np.float16)
    for k in range(2):
        Gs, G8f = _build_G(weights[k])
        G0c = Gs[0] @ np.diag(_YINIT)
        # pos-0 matrix: rhs is the w-window itself ([c;c;s;s] blocks)
        B0c = np.zeros((64, 64))
        G0r, G0i = G0c.real, G0c.imag
        for bout in range(4):
            re_out = bout in (0, 3)
            for j in range(DIM):
                col = bout * 16 + j
                if re_out:
                    B0c[0:16, col] = G0r[j]
                    B0c[32:48, col] = -G0i[j]
                else:
                    B0c[0:16, col] = G0i[j]
                    B0c[32:48, col] = G0r[j]
        base = k * K16COLS
        wm[:, base : base + 128] = (_STAGE_SCALE * _blockdiag2(B0c)).astype(
            np.float16
        )
        for p in range(1, 8):
            wm[:, base + p * 128 : base + (p + 1) * 128] = (
                _STAGE_SCALE * _blockdiag2(_wc_of(Gs[p], [0, 1, 2, 3]))
            ).astype(np.float16)
        wm[:, base + 1024 : base + 1088] = (
            _STAGE_SCALE * _blockdiag2(_wc_of(G8f, [0, 1]))
        ).astype(np.float16)
    wm[:, COL_ZL : COL_ZL + 16] = _zl2().astype(np.float16)
    return wm


def _zl2():
    """ZL2 (128, 16) packed 2-kernel measurement matrix. Row
    64k + 32m + 16h + i, col 8k + 4m + q = SIGMA[q, i]."""
    zl = np.zeros((128, 16), dtype=np.float32)
    for k in range(2):
        for m in range(IMGS_PER_CORE):
            for h in range(2):
                for q in range(IN_CH):
                    for i in range(DIM):
                        zl[64 * k + 32 * m + 16 * h + i, 8 * k + 4 * m + q] = _SIGMA[
                            q, i
                        ]
    return zl


def _build_x2(x):
    """x (16, 4, 64, 64) -> per-core pixel arrays (8, 4096):
    rows = [img0 ch0..3, img1 ch0..3]."""
    xf = np.ascontiguousarray(x, dtype=np.float32).reshape(B, IN_CH, NPIX)
    return [
        np.ascontiguousarray(xf[2 * c : 2 * c + 2].reshape(8, NPIX))
        for c in range(N_CORES)
    ]


# ---------------- custom fused DVE op: w = ((v^2 - 3/4) * v) * y ------------
_CUBE_OP = None


def _register_cube_mul():
    """Register the fused triple-angle multiply as a custom DVE op."""
    global _CUBE_OP
    if _CUBE_OP is not None:
        return _CUBE_OP
    import concourse.dve_ops as dve_ops

    for o in dve_ops.OPS:
        if o.name == "CUBE_MUL_ANT":
            _CUBE_OP = o
            return o
    from concourse.dve_ops import DveOp
    from concourse.dve_spec import C0, Spec, Src0, Src1, lower
    from concourse.dve_uop import DveOpSpec

    body = ((Src0 * Src0 - C0) * Src0) * Src1
    spec = Spec(
        body=body,
        reference=lambda in0, in1, c0, c1, c2: (
            ((in0.astype(np.float32) * in0 - c0) * in0) * in1
        ),
    )
    row = max(dve_ops._SUB_OPCODE_FOR_NAME.values()) + 1
    shas = {}
    for ver in ("v3", "v4"):
        uops = lower(spec, ver=ver)
        shas[ver] = DveOpSpec(
            name="CUBE_MUL_ANT", opcode=row, uops=uops, rd1_en=True
        ).sha(ver)
    op = DveOp("CUBE_MUL_ANT", spec, subdim=False, uops_sha=shas)
    dve_ops.OPS.append(op)
    dve_ops._SUB_OPCODE_FOR_NAME[op.name] = row
    dve_ops.CUSTOM_DVE_SPECS[op.name] = spec
    _CUBE_OP = op
    return op


# ---------------- device program ----------------
_PROGRAM_CACHE = {}


def _build_program(mm_dt):
    key = str(mm_dt)
    if key in _PROGRAM_CACHE:
        return _PROGRAM_CACHE[key]

    nc = bacc.Bacc("TRN2", target_bir_lowering=False, debug=False)
    x2_d = nc.dram_tensor("xpix", [8, NPIX], mm_dt, kind="ExternalInput").ap()
    wm_d = nc.dram_tensor(
        "wmats", [128, WM_COLS], mm_dt, kind="ExternalInput"
    ).ap()
    wm16_d = nc.dram_tensor(
        "wmats16", [128, WM16_COLS], mybir.dt.float16, kind="ExternalInput"
    ).ap()
    z_d = nc.dram_tensor(
        "zout", [2, 8, F], mybir.dt.float32, kind="ExternalOutput"
    ).ap()

    f32 = mybir.dt.float32
    CUBE = _register_cube_mul()

    # patch-row chunks: (py0, nrows)
    chunks = []
    py0 = 0
    while py0 < OH:
        chunks.append((py0, min(ROWCHUNK, OH - py0)))
        py0 += ROWCHUNK
    nch = len(chunks)

    with tile.TileContext(nc) as tc:
        from contextlib import ExitStack

        with ExitStack() as ctx:
            const_pool = ctx.enter_context(tc.tile_pool(name="const", bufs=1))
            m16_pool = ctx.enter_context(tc.tile_pool(name="m16", bufs=8))
            y16_pool = ctx.enter_context(tc.tile_pool(name="y16", bufs=8))
            sq_pool = ctx.enter_context(tc.tile_pool(name="sq", bufs=2))
            zs_pool = ctx.enter_context(tc.tile_pool(name="zs", bufs=2))
            a_pool = ctx.enter_context(tc.tile_pool(name="aps", bufs=2, space="PSUM"))
            # six single-buffer chain slots: each chain's state evolves
            # IN PLACE in one PSUM bank (TT(y_p) -> m -> MM -> y_{p+1} is
            # serial per chain, so double-buffering buys nothing).
            yp = [
                ctx.enter_context(tc.tile_pool(name=f"y{i}", bufs=1, space="PSUM"))
                for i in range(6)
            ]

            wm_sb = const_pool.tile([128, WM_COLS], mm_dt)
            nc.sync.dma_start(wm_sb[:], wm_d[:])
            wm16_sb = const_pool.tile([128, WM16_COLS], F16)
            nc.sync.dma_start(wm16_sb[:], wm16_d[:])
            x2_sb = const_pool.tile([8, NPIX], mm_dt)
            nc.sync.dma_start(x2_sb[:], x2_d[:])
            ones_sb = const_pool.tile([128, 512], f32)
            nc.vector.memset(ones_sb[:], 1.0)
            v_sb = const_pool.tile([128, NPIX], f32)
            w_sb = const_pool.tile([128, NPIX], mm_dt)
            wp16 = [
                const_pool.tile([128, WPCOLS], F16, name=f"wp16_{dx}")
                for dx in range(3)
            ]

            sgn_l = wm_sb[0:8, COL_SGN : COL_SGN + 128]
            z_l2 = wm16_sb[:, COL_ZL : COL_ZL + 16]
            bias_col = wm_sb[:, COL_BIAS : COL_BIAS + 1].bitcast(f32)

            def w16_ap(k, p):
                base = k * K16COLS
                if p == 0:
                    return wm16_sb[:, base : base + 128]
                if p < 8:
                    return wm16_sb[:, base + p * 128 : base + (p + 1) * 128]
                return wm16_sb[:, base + 1024 : base + 1088]

            def w_win(ci, pos):
                # fp32 shifted pixel-layout window (3 free dims)
                py0, nr = chunks[ci]
                dy, dx = divmod(pos, 3)
                ap = w_sb[:].rearrange("p (r c) -> p r c", c=HW)
                return ap[:, py0 + dy : py0 + dy + nr, dx : dx + OH]

            def wp_win(ci, pos):
                # fp16 contiguous patch-layout window
                py0, nr = chunks[ci]
                dy, dx = divmod(pos, 3)
                r0 = (py0 + dy) * OH
                return wp16[dx][:, r0 : r0 + nr * OH]

            # ---- prologue: g -> v -> w over the full pixel grid ----
            for j in range(NPIX // 512):
                g_ps = a_pool.tile([128, 512], f32, tag="aps", name=f"g{j}")
                nc.tensor.matmul(
                    g_ps[:],
                    sgn_l,
                    x2_sb[:, j * 512 : (j + 1) * 512],
                    start=True,
                    stop=True,
                )
                nc.scalar.activation(
                    v_sb[:, j * 512 : (j + 1) * 512],
                    g_ps[:],
                    mybir.ActivationFunctionType.Sin,
                    bias=bias_col,
                    scale=1.0,
                )
                nc.vector._custom_dve(
                    CUBE,
                    out=w_sb[:, j * 512 : (j + 1) * 512],
                    in0=v_sb[:, j * 512 : (j + 1) * 512],
                    in1=ones_sb[:],
                    s0=0.75,
                )
            # fp16 patch-layout pre-gathered copies of w (one per dx shift);
            # contiguous reads keep the 2x DVE mode clean.  Split into row
            # halves so early chunks unblock sooner.
            w_pix = w_sb[:].rearrange("p (r c) -> p r c", c=HW)
            for dx in range(3):
                for h in range(2):
                    nc.vector.tensor_copy(
                        wp16[dx][:, h * 32 * OH : (h + 1) * 32 * OH],
                        w_pix[:, h * 32 : (h + 1) * 32, dx : dx + OH].bitcast(f32),
                    )

            # PE warm-up: a back-to-back burst of dummy matmuls flips the
            # HAM clock gate to 8/8 (2.4 GHz) before the state chains start.
            wu = a_pool.tile([128, 512], f32, tag="aps", name="wu")
            for wi in range(12):
                nc.tensor.matmul(
                    wu[:, 0:128],
                    ones_sb[:, 0:128],
                    ones_sb[:, 0:128],
                    start=True,
                    stop=True,
                )

            chain = {}

            def do_state(ci, pos):
                py0, nr = chunks[ci]
                C = nr * OH
                cur = chain.setdefault(ci, [None, None])
                for k in range(2):
                    if pos == 0:
                        pool = yp[(2 * ci + k) % 6]
                        cur[k] = pool.tile(
                            [128, 512],
                            f32,
                            tag=f"y{(2 * ci + k) % 6}",
                            name=f"y{ci}_{k}",
                        )
                    # pos 8: both kernels' final [Re;Im] states share k0's
                    # tile (k0 -> rows 0:64, k1 -> rows 64:128).
                    if pos == 8:
                        out_ap = cur[0][64 * k : 64 * k + 64, :C]
                    else:
                        out_ap = cur[k][:, :C]
                    if pos == 0:
                        nc.tensor.matmul(
                            out_ap, w16_ap(k, 0), wp_win(ci, 0),
                            start=True, stop=True,
                        )
                    else:
                        mode = _mode(pos, k)
                        m16 = m16_pool.tile([128, 512], F16, tag="m16")
                        if mode == "A":
                            nc.vector.tensor_mul(
                                m16[:, :C], w_win(ci, pos), cur[k][:, :C]
                            )
                        else:
                            y16 = y16_pool.tile([128, 512], F16, tag="y16")
                            nc.scalar.copy(y16[:, :C], cur[k][:, :C])
                            eng = nc.vector if mode == "B" else nc.gpsimd
                            eng.tensor_mul(
                                m16[:, :C], wp_win(ci, pos), y16[:, :C]
                            )
                        nc.tensor.matmul(
                            out_ap, w16_ap(k, pos), m16[:, :C],
                            start=True, stop=True,
                        )

            def do_meas(ci, _pos):
                py0, nr = chunks[ci]
                C = nr * OH
                c0 = py0 * OH
                y8 = chain.pop(ci)[0]  # packed k0/k1 final states
                sq = sq_pool.tile([128, 512], F16, tag="sq")
                nc.scalar.activation(
                    sq[:, :C], y8[:, :C], mybir.ActivationFunctionType.Square
                )
                zq = a_pool.tile([128, 512], f32, tag="aps", name=f"zq{ci}")
                nc.tensor.matmul(zq[0:16, :C], z_l2, sq[:, :C], start=True, stop=True)
                zs = zs_pool.tile([16, 512], f32, tag="zs")
                nc.scalar.copy(zs[0:16, :C], zq[0:16, :C])
                nc.sync.dma_start(
                    z_d[:, :, c0 : c0 + C].rearrange("a b c -> (a b) c"),
                    zs[0:16, :C],
                )

            # merged time-ordered emission: chains pipelined across chunks
            OFF = 3.5
            events = []
            for ci in range(nch):
                t0 = ci * OFF
                for pos in range(NPOS):
                    events.append((t0 + pos, 2, ci, pos, do_state))
                events.append((t0 + NPOS, 3, ci, 0, do_meas))
            events.sort(key=lambda e: (e[0], e[1], e[2]))
            for _t, _kind, ci, pos, fn in events:
                fn(ci, pos)

    nc.compile()
    _PROGRAM_CACHE[key] = nc
    return nc


# ---------------- entry point ----------------
def kernel(x, weights):
    x = np.asarray(x, dtype=np.float32)
    weights = np.asarray(weights, dtype=np.float32)
    wm = _build_wmats(weights)
    wm16 = _build_wmats16(weights)
    x2s = _build_x2(x)

    nc = _build_program(MM_DT)
    in_maps = [
        {"xpix": x2s[c], "wmats": wm, "wmats16": wm16} for c in range(N_CORES)
    ]
    res = run_bass_kernel_spmd(nc, in_maps, list(range(N_CORES)))

    out = np.zeros((B, 2 * IN_CH, OH, OH), dtype=np.float32)
    for c in range(N_CORES):
        z = np.asarray(res.results[c]["zout"])  # (2, 8, F)
        for k in range(2):
            for chunk in range(2):
                b = 2 * c + chunk
                out[b, k * 4 : k * 4 + 4] = z[k, chunk * 4 : chunk * 4 + 4].reshape(
                    IN_CH, OH, OH
                )
    return out
